# revision 6
# baseline (speedup 1.0000x reference)
"""Trainium2 Bass kernel for nn_MemoryEfficientBSpline — PE 64x64 tiled, v10 (k-outer, natural banks).

Hinge-basis math (see kernel_v7 docstring). The PE runs in 64x64 tiling mode:
four independent 64x64 tiles (T0, T2, T8, T10) stream concurrently for ~2x
column throughput over the block-diag fold.

Layout per core: x folded [128, 2, 9216] f16 (axis 1: column region A/B; rows
0-63 half-A channels, 64-127 half-B). Region A is computed by T0 (half-A ->
psum rows 0-63) + T10 (half-B -> rows 64-127) = normal orientation; region B
by T2 (half-A -> rows 64-127) + T8 (half-B -> rows 0-63) = swapped halves,
un-swapped on the host during reassembly.

PSUM: [128, 1024] tiles (2 banks); a pair fills one in two staggered
half-phases (partner tiles on opposite banks, then swap) so no two PE tiles
touch one bank simultaneously, yet each bank finishes as a full [128, 512]
y-block.

x is pre-clipped to [-1,1] on the host so plane_0 == x (4 DVE clips, not 5).
Queues: x-in on SP (one 2-region AP DMA per fetch), y-out on the gpsimd SWDGE
queue (one [128, 2, 3072] DMA per fetch), weights/bias + most evicts on ACT,
~2 evicts per body on DVE. All f16 except PSUM/bias.
"""
import numpy as np
from contextlib import ExitStack

import concourse.bass as bass
import concourse.tile as tile
from concourse import bacc, mybir
from concourse.bass_utils import run_bass_kernel_spmd

B, IN_DIM, H, W = 8, 64, 192, 192
OUT_DIM = 64
P_TOT = H * W            # 36864
HALF = P_TOT // 2        # 18432
NPART = 128
REG = HALF // 2          # 9216 per region
CHUNK = 512
PHASE = 2 * CHUNK        # 1024 (one [128,1024] psum tile per region per phase)
FETCH = 3072             # columns fetched per region per step
N_FETCH = REG // FETCH   # 3
PH_PER_FETCH = FETCH // PHASE  # 3
NK = 5

_f32 = mybir.dt.float32
_f16 = mybir.dt.float16
_Alu = mybir.AluOpType
_Act = mybir.ActivationFunctionType

_cached = None


def _build_module(n_reps=1):
    nc = bacc.Bacc("TRN2", target_bir_lowering=False, debug=False,
                   enable_asserts=False, num_devices=8)

    x_t = nc.dram_tensor("x", (NPART, 2, REG), _f16, kind="ExternalInput")
    w_t = nc.dram_tensor("wts", (NPART, NK * 64), _f16, kind="ExternalInput")
    b_t = nc.dram_tensor("bias", (NPART, 1), _f32, kind="ExternalInput")
    y_t = nc.dram_tensor("y", (NPART, 2, REG), _f16, kind="ExternalOutput")

    lo = slice(0, 64)
    hi = slice(64, 128)

    with tile.TileContext(nc) as tc, ExitStack() as ctx:
        cpool = ctx.enter_context(tc.tile_pool(name="const", bufs=1))
        xpool = ctx.enter_context(tc.tile_pool(name="xin", bufs=3))
        ppool = ctx.enter_context(tc.tile_pool(name="planes", bufs=3))
        opool = ctx.enter_context(tc.tile_pool(name="oslab", bufs=2))
        psum = ctx.enter_context(tc.tile_pool(name="acc", bufs=4, space="PSUM"))

        wts = cpool.tile([NPART, NK * 64], _f16)
        nc.scalar.dma_start(wts[:], w_t[:])
        bias = cpool.tile([NPART, 1], _f32)
        nc.scalar.dma_start(bias[:], b_t[:])

        # PE warmup in 64x64 mode (same tiling mode as the body): ramp the
        # p-states during the pipeline fill.
        wsrc = cpool.tile([NPART, CHUNK], _f16)
        nc.vector.memset(wsrc[:], 0.0)
        wp = psum.tile([NPART, PHASE], _f32, tag="acc", name="acc")
        for r in range(12):
            nc.tensor.matmul(wp[lo, :CHUNK], wsrc[lo, :64], wsrc[lo, :],
                             start=True, stop=True)

        def body():
            dve_evicts = 0
            for f in range(N_FETCH):
                a0 = f * FETCH
                xt = xpool.tile([NPART, 2, FETCH], _f16)
                if f == 0:
                    # Quartered first fetch: planes can start ~3us sooner.
                    for q in range(PH_PER_FETCH):
                        qs = slice(q * PHASE, (q + 1) * PHASE)
                        nc.sync.dma_start(xt[:, :, qs], x_t[:, :, qs])
                else:
                    nc.sync.dma_start(xt[:], x_t[:, :, a0:a0 + FETCH])

                # planes: p0 == xt; p1..p4 = clip(x, t_k, 1) on DVE (f16 4x).
                # First fetch in phase-size pieces to shorten the fill.
                planes = [xt]
                for k in range(1, NK):
                    pk = ppool.tile([NPART, 2, FETCH], _f16, tag=f"p{k}",
                                    name=f"p{k}")
                    planes.append(pk)
                pieces = PH_PER_FETCH if f == 0 else 1
                pw = FETCH // pieces
                for q in range(pieces):
                    qs = slice(q * pw, (q + 1) * pw)
                    for k in range(1, NK):
                        tk = -1.0 + 0.4 * k
                        nc.vector.tensor_scalar(planes[k][:, :, qs],
                                                xt[:, :, qs], 1.0, tk,
                                                _Alu.min, _Alu.max)

                ot = opool.tile([NPART, 2, FETCH], _f16)
                for ph in range(PH_PER_FETCH):
                    ca = ph * PHASE
                    accA = psum.tile([NPART, PHASE], _f32, tag="acc", name="acc")
                    accB = psum.tile([NPART, PHASE], _f32, tag="acc", name="acc")
                    # k-outer, both 512-chunks per tile per k: halves LDW
                    # traffic. Partner tiles write opposite partition-halves
                    # of the same banks concurrently (validated safe on HW).
                    for k in range(NK):
                        wk = wts[:, k * 64:(k + 1) * 64]
                        pk = planes[k]
                        for c in range(2):
                            cb = slice(c * CHUNK, (c + 1) * CHUNK)
                            sc = slice(ca + c * CHUNK, ca + (c + 1) * CHUNK)
                            nc.tensor.matmul(accA[lo, cb], wk[lo], pk[lo, 0, sc],
                                             start=(k == 0), stop=(k == NK - 1),
                                             skip_group_check=True)
                        for c in range(2):
                            cb = slice(c * CHUNK, (c + 1) * CHUNK)
                            sc = slice(ca + c * CHUNK, ca + (c + 1) * CHUNK)
                            nc.tensor.matmul(accA[hi, cb], wk[hi], pk[hi, 0, sc],
                                             start=(k == 0), stop=(k == NK - 1),
                                             skip_group_check=True)
                        for c in range(2):
                            cb = slice(c * CHUNK, (c + 1) * CHUNK)
                            sc = slice(ca + c * CHUNK, ca + (c + 1) * CHUNK)
                            nc.tensor.matmul(accB[hi, cb], wk[lo], pk[lo, 1, sc],
                                             start=(k == 0), stop=(k == NK - 1),
                                             skip_group_check=True)
                        for c in range(2):
                            cb = slice(c * CHUNK, (c + 1) * CHUNK)
                            sc = slice(ca + c * CHUNK, ca + (c + 1) * CHUNK)
                            nc.tensor.matmul(accB[lo, cb], wk[hi], pk[hi, 1, sc],
                                             start=(k == 0), stop=(k == NK - 1),
                                             skip_group_check=True)

                    # Evict both psum tiles into the fetch-wide ot. Mostly on
                    # ACT; ~2 per body on DVE to keep ACT under the PE pace.
                    ps = slice(ca, ca + PHASE)
                    nc.scalar.activation(ot[:, 0, ps], accA[:], _Act.Identity,
                                         bias=bias[:], scale=1.0)
                    if dve_evicts < 2:
                        dve_evicts += 1
                        nc.vector.tensor_scalar(ot[:, 1, ps], accB[:],
                                                bias[:], None, _Alu.add)
                    else:
                        nc.scalar.activation(ot[:, 1, ps], accB[:],
                                             _Act.Identity,
                                             bias=bias[:], scale=1.0)

                # One SWDGE y store per fetch; the last fetch ships per-phase
                # pieces across Pool/SP/ACT so the drain tail is ~one phase.
                if f < N_FETCH - 1:
                    nc.gpsimd.dma_start(y_t[:, :, a0:a0 + FETCH], ot[:])
                else:
                    qs0 = slice(a0, a0 + PHASE)
                    nc.gpsimd.dma_start(y_t[:, :, qs0], ot[:, :, :PHASE])
                    qs1 = slice(a0 + PHASE, a0 + 2 * PHASE)
                    nc.sync.dma_start(y_t[:, :, qs1], ot[:, :, PHASE:2 * PHASE])
                    qs2 = slice(a0 + 2 * PHASE, a0 + FETCH)
                    nc.scalar.dma_start(y_t[:, :, qs2], ot[:, :, 2 * PHASE:])

        if n_reps == 1:
            body()
        else:
            with tc.For_i(0, n_reps, 1):
                body()

    nc.compile()
    return nc


def _get_module():
    global _cached
    if _cached is None:
        _cached = _build_module()
    return _cached


def _prep_inputs(x, coef):
    x = np.asarray(x, dtype=np.float32)
    c = np.asarray(coef, dtype=np.float64)
    d = np.diff(c, axis=-1)
    beta = np.concatenate([d[..., :1], np.diff(d, axis=-1)], axis=-1)
    Wk64 = 2.5 * beta
    Wk = Wk64.astype(np.float32)
    alpha = (c[..., 0].sum(axis=2) + Wk64[..., 0].sum(axis=2)
             + sum((1.0 - 0.4 * k) * Wk64[..., k].sum(axis=2) for k in (1, 2, 3, 4))
             ).astype(np.float32)

    in_maps = []
    for b in range(B):
        xb = np.clip(x[b].reshape(IN_DIM, P_TOT), -1.0, 1.0)  # pre-clip: p0 == x
        x_f = np.concatenate([xb[:, :HALF], xb[:, HALF:]], axis=0)  # [128, HALF]
        lhsT = np.einsum('oik->kio', Wk[b])            # [5, i, o]
        wts = np.concatenate([lhsT, lhsT], axis=1)     # [5, 128, 64]
        wts = np.transpose(wts, (1, 0, 2)).reshape(NPART, NK * 64)
        bias_b = np.tile(alpha[b], 2).reshape(NPART, 1).astype(np.float32)
        in_maps.append({
            "x": np.ascontiguousarray(x_f.astype(np.float16).reshape(NPART, 2, REG)),
            "wts": np.ascontiguousarray(wts.astype(np.float16)),
            "bias": bias_b,
        })
    return in_maps


def _assemble(results):
    out = np.empty((B, OUT_DIM, H, W), dtype=np.float32)
    for b in range(B):
        y_f = results[b]["y"].astype(np.float32).reshape(NPART, HALF)
        yb = y_f.copy()
        yb[:OUT_DIM, REG:] = y_f[OUT_DIM:, REG:]   # un-swap region B halves
        yb[OUT_DIM:, REG:] = y_f[:OUT_DIM, REG:]
        out[b] = np.concatenate([yb[:OUT_DIM], yb[OUT_DIM:]], axis=1).reshape(OUT_DIM, H, W)
    return out


def run(x, coef, **spmd_kwargs):
    nc = _get_module()
    in_maps = _prep_inputs(x, coef)
    res = run_bass_kernel_spmd(nc, in_maps, core_ids=list(range(8)), **spmd_kwargs)
    return _assemble(res.results), res


def kernel(x, coef):
    out, _ = run(x, coef)
    return out
